# revision 26
# baseline (speedup 1.0000x reference)
"""MoE layer (8 experts, top-2, SwiGLU FFN) on 8 Trainium2 NeuronCores.

Sharding: expert-parallel. Core c holds expert c's weights and computes
  partial_c[d, t] = combine[t, c] * FFN_c(xn)[t, d]   (transposed layout)
for all 4096 tokens; a ReduceScatter(add) over the 8 cores then splits the
summed transposed output row-wise, and the host concatenates + transposes.

On-core pipeline (activations kept d-major, i.e. transposed, so weight
matrices act as pre-transposed stationary operands):
  1. RMSNorm + transpose fused: xn^T chunk = x_tile.T @ diag(1/rms) scaled
     by rms_w per-partition on PSUM eviction (PE transpose trick).
  2. Router in fp32 on PE; top-2 via two masked max-reductions; renorm
     weights w1 = sigmoid(l1 - l2), w2 = 1 - w1 (equivalent to softmax
     top-2 renormalization); combine column broadcast across partitions
     with a ones @ diag(c) matmul.
  3. SwiGLU FFN in bf16 (fp32 PSUM accumulation), biases fused into
     ACT/DVE eviction ops.
"""

import contextlib
import ctypes
import os
import sys
import types

import numpy as np

# ---------------------------------------------------------------------------
# Optional NTFF profiling shim: antenv.axon_hooks is missing in this image;
# recreate it around libaxon's C ABI so trace=True can report HW exec time.
# ---------------------------------------------------------------------------


def _install_axon_hooks_shim(so_path="/opt/axon/libaxon_pjrt.so"):
    if "antenv.axon_hooks" in sys.modules:
        return
    mod = types.ModuleType("antenv.axon_hooks")
    mod._hook = None

    def set_axon_ntff_profile_hook(h):
        mod._hook = h

    def get_axon_ntff_profile_hook():
        return mod._hook

    mod.set_axon_ntff_profile_hook = set_axon_ntff_profile_hook
    mod.get_axon_ntff_profile_hook = get_axon_ntff_profile_hook
    sys.modules["antenv.axon_hooks"] = mod
    try:
        import antenv

        antenv.axon_hooks = mod
    except ImportError:
        pass
    try:
        lib = ctypes.CDLL(so_path)
    except OSError:
        return
    if not hasattr(lib, "axon_start_nrt_profile"):
        return
    lib.axon_start_nrt_profile.argtypes = [
        ctypes.POINTER(ctypes.c_int64),
        ctypes.c_size_t,
    ]
    lib.axon_start_nrt_profile.restype = ctypes.c_int64
    lib.axon_stop_nrt_profile.argtypes = [ctypes.c_char_p]
    lib.axon_stop_nrt_profile.restype = ctypes.c_int64

    @contextlib.contextmanager
    def _hook(output_dir, device_ids):
        import jax

        jax.devices()
        if device_ids:
            ids = (ctypes.c_int64 * len(device_ids))(*device_ids)
            rc = lib.axon_start_nrt_profile(ids, len(device_ids))
        else:
            rc = lib.axon_start_nrt_profile(None, 0)
        if rc != 0:
            raise RuntimeError(f"axon_start_nrt_profile rc={rc}")
        try:
            yield
        finally:
            n = lib.axon_stop_nrt_profile(str(output_dir).encode())
            print(f"profile: {n} file(s) written to {output_dir}", file=sys.stderr)

    set_axon_ntff_profile_hook(_hook)


_install_axon_hooks_shim()

import concourse.bacc as bacc
import concourse.bass as bass
import concourse.mybir as mybir
import concourse.tile as tile
from concourse.bass_utils import run_bass_kernel_spmd
from concourse.masks import make_identity

AF = mybir.ActivationFunctionType
OP = mybir.AluOpType
F32 = mybir.dt.float32
BF16 = mybir.dt.bfloat16

N_CORES = 8
CORE_IDS = list(range(N_CORES))

B, T, D, F, E = 2, 2048, 1024, 4096, 8
NTOK = B * T            # 4096 tokens
TT = 128                # token tile (phase 1)
NTT = NTOK // TT        # 32
FT = 512                # ffn token tile (phase 2)
NFT = NTOK // FT        # 8
KD = D // 128           # 8 contraction chunks over d
MF = F // 128           # 32 f chunks
EPS = 1e-7
MM_DT = BF16            # FFN matmul dtype

LAST_EXEC_NS = None
_CACHED = None


def _build():
    nc = bacc.Bacc(
        "TRN2", target_bir_lowering=False, debug=False, num_devices=N_CORES
    )

    x_d = nc.dram_tensor("x", [NTOK, D], F32, kind="ExternalInput")
    rmsw_d = nc.dram_tensor("rms_w", [D], F32, kind="ExternalInput")
    rw_d = nc.dram_tensor("router_w", [E, D], F32, kind="ExternalInput")
    rb_d = nc.dram_tensor("router_b", [E], F32, kind="ExternalInput")
    esel_d = nc.dram_tensor("esel", [E], F32, kind="ExternalInput")
    W1_d = nc.dram_tensor("W1", [D, F], F32, kind="ExternalInput")
    b1_d = nc.dram_tensor("b1", [F], F32, kind="ExternalInput")
    W2_d = nc.dram_tensor("W2", [F, D], F32, kind="ExternalInput")
    b2_d = nc.dram_tensor("b2", [D], F32, kind="ExternalInput")
    W3_d = nc.dram_tensor("W3", [D, F], F32, kind="ExternalInput")
    b3_d = nc.dram_tensor("b3", [F], F32, kind="ExternalInput")

    xnb_ds = [
        nc.dram_tensor(f"xnb{i}", [D, FT], BF16) for i in range(NFT)
    ]  # normalized, transposed, bf16, one tensor per ffn supertile
    NRS = int(os.environ.get("BASSMOE_NRS", "8"))  # reduce-scatter chunks
    RSTOK = NTOK // NRS
    outT_ds = [nc.dram_tensor(f"outT{j}", [D, RSTOK], F32) for j in range(NRS)]
    rs_ds = [
        nc.dram_tensor(f"rs{j}", [D // N_CORES, RSTOK], F32) for j in range(NRS)
    ]
    W1b_ds = [nc.dram_tensor(f"W1b{m}", [128, KD * 128], BF16) for m in range(MF)]
    W3b_ds = [nc.dram_tensor(f"W3b{m}", [128, KD * 128], BF16) for m in range(MF)]
    W2b_ds = [nc.dram_tensor(f"W2b{m}", [128, MF * 128], BF16) for m in range(KD)]
    out_ext = nc.dram_tensor("outp", [D // N_CORES, NTOK], F32, kind="ExternalOutput")

    with tile.TileContext(nc) as tc:
        const_ctx = contextlib.ExitStack()
        const = const_ctx.enter_context(tc.tile_pool(name="const", bufs=1))
        with contextlib.ExitStack() as ctx:
            ph1 = ctx.enter_context(tc.tile_pool(name="ph1", bufs=5))
            xnp = ctx.enter_context(tc.tile_pool(name="xnp", bufs=4))
            ps1 = ctx.enter_context(
                tc.tile_pool(name="ps1", bufs=2, space="PSUM")
            )

            # ---- constants ----
            ident = const.tile([128, 128], F32)
            make_identity(nc, ident[:])
            ones_t = const.tile([128, 128], F32)
            nc.vector.memset(ones_t[:], 1.0)

            rw_sb = const.tile([E, D], F32)
            nc.sync.dma_start(out=rw_sb[:], in_=rw_d.ap())
            rwT = const.tile([128, KD, E], F32)
            for k in range(KD):
                pt = ps1.tile([128, 128], F32, name="ps_big")[:, :E]
                nc.tensor.transpose(
                    pt[:], rw_sb[:, k * 128 : (k + 1) * 128], ident[:E, :E]
                )
                nc.vector.tensor_copy(out=rwT[:, k, :], in_=pt[:])

            def load_col_chunks(dram, n, name):
                # [n*128] dram vector -> [128, n] sbuf tile, col j = chunk j
                raw = const.tile([n, 128], F32, name=name + "_raw")
                nc.sync.dma_start(
                    out=raw[:], in_=dram.ap().rearrange("(m p) -> m p", p=128)
                )
                pt = ps1.tile([128, 128], F32, name="ps_big")[:, :n]
                nc.tensor.transpose(pt[:], raw[:], ident[:n, :n])
                out = const.tile([128, n], F32, name=name)
                nc.vector.tensor_copy(out=out[:], in_=pt[:])
                return out

            b1T = load_col_chunks(b1_d, MF, "b1T")
            b3T = load_col_chunks(b3_d, MF, "b3T")
            b2T = load_col_chunks(b2_d, KD, "b2T")
            rmswT = load_col_chunks(rmsw_d, KD, "rmswT")

            rb_row = const.tile([1, E], F32)
            nc.sync.dma_start(
                out=rb_row[:], in_=rb_d.ap().rearrange("(a e) -> a e", a=1)
            )
            rbB = const.tile([128, E], F32)
            nc.gpsimd.partition_broadcast(rbB[:], rb_row[:1, :])
            esel_row = const.tile([1, E], F32)
            nc.sync.dma_start(
                out=esel_row[:], in_=esel_d.ap().rearrange("(a e) -> a e", a=1)
            )
            eselB = const.tile([128, E], F32)
            nc.gpsimd.partition_broadcast(eselB[:], esel_row[:1, :])

            cbBs = [
                const.tile([128, FT], F32, name=f"cbB{i}") for i in range(NFT)
            ]  # combine col bcast over partitions, one tile per ffn supertile

            # ---- phase 1: rmsnorm+transpose, router, combine ----
            # grouped passes (8 tiles each): stats batched per group so the
            # first transposes issue after ~8 tiles instead of all 32
            SSQ = const.tile([128, NTT], F32)
            MSQ = const.tile([128, NTT], F32)
            RMS = const.tile([128, NTT], F32)
            INV = const.tile([128, NTT], F32)
            EQ1 = const.tile([128, NTT, E], F32)
            EQ2 = const.tile([128, NTT, E], F32)
            DLT = const.tile([128, NTT], F32)
            GRP = 8
            for g in range(0, NTT, GRP):
                gs = slice(g, g + GRP)
                for t in range(g, g + GRP):
                    ts = slice(t * TT, (t + 1) * TT)
                    x_t = ph1.tile([128, D], F32, name="x_t")
                    nc.sync.dma_start(out=x_t[:], in_=x_d.ap()[ts, :])
                    sq = ph1.tile([128, D], F32, name="sq")
                    nc.vector.scalar_tensor_tensor(
                        out=sq[:], in0=x_t[:], scalar=1.0, in1=x_t[:],
                        op0=OP.bypass, op1=OP.mult, accum_out=SSQ[:, t : t + 1],
                    )
                nc.vector.tensor_scalar(
                    out=MSQ[:, gs], in0=SSQ[:, gs], scalar1=1.0 / D,
                    scalar2=EPS, op0=OP.mult, op1=OP.add,
                )
                nc.scalar.sqrt(RMS[:, gs], MSQ[:, gs])
                nc.vector.reciprocal(INV[:, gs], RMS[:, gs])

                for t in range(g, g + GRP):
                    ts = slice(t * TT, (t + 1) * TT)
                    x_t = ph1.tile([128, D], F32, name="x_t")
                    nc.sync.dma_start(out=x_t[:], in_=x_d.ap()[ts, :])
                    dg = ph1.tile([128, 128], F32, name="dg")
                    nc.vector.tensor_scalar_mul(
                        dg[:], ident[:], INV[:, t : t + 1]
                    )

                    xn_t = xnp.tile([128, KD, 128], F32, name="xn_t")
                    xnb_t = xnp.tile([128, KD, 128], BF16, name="xnb_t")
                    tau, col = divmod(t, FT // TT)
                    for k in range(KD):
                        pxn = ps1.tile([128, 128], F32, name="ps_big")
                        nc.tensor.matmul(
                            pxn[:], lhsT=x_t[:, k * 128 : (k + 1) * 128],
                            rhs=dg[:], start=True, stop=True,
                        )
                        nc.vector.tensor_scalar_mul(
                            xn_t[:, k, :], pxn[:], rmswT[:, k : k + 1]
                        )
                        nc.vector.tensor_copy(
                            out=xnb_t[:, k, :], in_=xn_t[:, k, :]
                        )
                        nc.sync.dma_start(
                            out=xnb_ds[tau].ap()[
                                k * 128 : (k + 1) * 128,
                                col * TT : (col + 1) * TT,
                            ],
                            in_=xnb_t[:, k, :],
                        )

                    lg_ps = ps1.tile([128, 128], F32, name="ps_big")[:, :E]
                    for k in range(KD):
                        nc.tensor.matmul(
                            lg_ps[:], lhsT=xn_t[:, k, :], rhs=rwT[:, k, :],
                            start=(k == 0), stop=(k == KD - 1),
                        )
                    lg = ph1.tile([128, E], F32, name="lg")
                    nc.vector.tensor_add(out=lg[:], in0=lg_ps[:], in1=rbB[:])
                    l1 = ph1.tile([128, 1], F32, name="l1")
                    nc.vector.tensor_reduce(
                        l1[:], lg[:], axis=mybir.AxisListType.X, op=OP.max
                    )
                    nc.vector.tensor_scalar(
                        out=EQ1[:, t, :], in0=lg[:], scalar1=l1[:],
                        scalar2=None, op0=OP.is_equal,
                    )
                    lg2 = ph1.tile([128, E], F32, name="lg2")
                    nc.vector.scalar_tensor_tensor(
                        out=lg2[:], in0=EQ1[:, t, :], scalar=-1e30, in1=lg[:],
                        op0=OP.mult, op1=OP.add,
                    )
                    l2 = ph1.tile([128, 1], F32, name="l2")
                    nc.vector.tensor_reduce(
                        l2[:], lg2[:], axis=mybir.AxisListType.X, op=OP.max
                    )
                    nc.vector.tensor_scalar(
                        out=EQ2[:, t, :], in0=lg2[:], scalar1=l2[:],
                        scalar2=None, op0=OP.is_equal,
                    )
                    nc.vector.tensor_sub(
                        out=DLT[:, t : t + 1], in0=l1[:], in1=l2[:]
                    )

            # pass C: batched sigmoid for the top-2 renorm weights
            WA = const.tile([128, NTT], F32)
            nc.scalar.activation(WA[:], DLT[:], AF.Sigmoid)
            WB = const.tile([128, NTT], F32)
            nc.vector.tensor_scalar(
                out=WB[:], in0=WA[:], scalar1=-1.0, scalar2=1.0,
                op0=OP.mult, op1=OP.add,
            )

            # pass D: combine column for this expert, broadcast over partitions
            for t in range(NTT):
                tau, col = divmod(t, FT // TT)
                tmp = ph1.tile([128, E], F32, name="tmp")
                nc.vector.tensor_scalar_mul(tmp[:], EQ2[:, t, :], WB[:, t : t + 1])
                cmb = ph1.tile([128, E], F32, name="cmb")
                nc.vector.scalar_tensor_tensor(
                    out=cmb[:], in0=EQ1[:, t, :], scalar=WA[:, t : t + 1],
                    in1=tmp[:], op0=OP.mult, op1=OP.add,
                )
                cmb2 = ph1.tile([128, E], F32, name="cmb2")
                c_col = ph1.tile([128, 1], F32, name="c_col")
                nc.vector.scalar_tensor_tensor(
                    out=cmb2[:], in0=cmb[:], scalar=1.0, in1=eselB[:],
                    op0=OP.bypass, op1=OP.mult, accum_out=c_col[:],
                )
                dgc = ph1.tile([128, 128], F32, name="dgc")
                nc.vector.tensor_scalar_mul(dgc[:], ident[:], c_col[:])
                cb_ps = ps1.tile([128, 128], F32, name="ps_big")
                nc.tensor.matmul(
                    cb_ps[:], lhsT=ones_t[:], rhs=dgc[:], start=True, stop=True
                )
                nc.vector.tensor_copy(
                    out=cbBs[tau][:, col * TT : (col + 1) * TT], in_=cb_ps[:]
                )

        # ---- phase 2: SwiGLU FFN in bf16 ----
        with contextlib.ExitStack() as ctx:
            xn2 = ctx.enter_context(tc.tile_pool(name="xn2", bufs=3))
            wp = ctx.enter_context(tc.tile_pool(name="wp", bufs=5))
            wpr = ctx.enter_context(tc.tile_pool(name="wpr", bufs=1))
            hp = ctx.enter_context(tc.tile_pool(name="hp", bufs=1))
            op_ = ctx.enter_context(tc.tile_pool(name="op", bufs=4))
            psA = ctx.enter_context(
                tc.tile_pool(name="psA", bufs=2, space="PSUM")
            )
            psB = ctx.enter_context(
                tc.tile_pool(name="psB", bufs=2, space="PSUM")
            )

            W1_r = W1_d.ap().rearrange("(k p) f -> p k f", p=128)
            W3_r = W3_d.ap().rearrange("(k p) f -> p k f", p=128)
            W2_r = W2_d.ap().rearrange("(k p) f -> p k f", p=128)

            grp_raw = {}

            def load_w(tau, raw_src, cache_d, m, nk, rtag, btag, gw=1):
                # tau 0: load fp32 (gw m-chunks per DMA for bigger segments),
                # cast per chunk, save bf16 for later taus
                wb = wp.tile([128, nk * 128], MM_DT, name=btag)
                if tau == 0:
                    r = m % gw
                    if r == 0:
                        wr = wpr.tile([128, nk, gw * 128], F32, name=rtag)
                        nc.sync.dma_start(
                            out=wr[:],
                            in_=raw_src[:, :, m * 128 : (m + gw) * 128],
                        )
                        grp_raw[rtag] = wr
                    wr = grp_raw[rtag]
                    nc.vector.tensor_copy(
                        out=wb[:].rearrange("p (k c) -> p k c", k=nk),
                        in_=wr[:, :, r * 128 : (r + 1) * 128],
                    )
                    nc.sync.dma_start(out=cache_d.ap(), in_=wb[:])
                else:
                    nc.sync.dma_start(out=wb[:], in_=cache_d.ap())
                return wb

            for tau in range(NFT):
                xb = xn2.tile([128, KD, FT], MM_DT, name="xb")
                nc.sync.dma_start(
                    out=xb[:],
                    in_=xnb_ds[tau].ap().rearrange("(k p) n -> p k n", p=128),
                )

                h_t = hp.tile([128, MF, FT], MM_DT, name="h_t")
                for m in range(MF):
                    w1b = load_w(tau, W1_r, W1b_ds[m], m, KD, "w1r", "w1b", gw=2)
                    w3b = load_w(tau, W3_r, W3b_ds[m], m, KD, "w3r", "w3b", gw=2)

                    p1 = psA.tile([128, FT], F32, name="p1")
                    p3 = psA.tile([128, FT], F32, name="p3")
                    for k in range(KD):
                        ks = slice(k * 128, (k + 1) * 128)
                        nc.tensor.matmul(
                            p1[:], lhsT=w1b[:, ks], rhs=xb[:, k, :],
                            start=(k == 0), stop=(k == KD - 1),
                        )
                    for k in range(KD):
                        ks = slice(k * 128, (k + 1) * 128)
                        nc.tensor.matmul(
                            p3[:], lhsT=w3b[:, ks], rhs=xb[:, k, :],
                            start=(k == 0), stop=(k == KD - 1),
                        )
                    h1s = op_.tile([128, FT], MM_DT, name="h1s")
                    nc.scalar.activation(
                        h1s[:], p1[:], AF.Silu, bias=b1T[:, m : m + 1]
                    )
                    nc.vector.scalar_tensor_tensor(
                        out=h_t[:, m, :], in0=p3[:], scalar=b3T[:, m : m + 1],
                        in1=h1s[:], op0=OP.add, op1=OP.mult,
                    )

                for m2 in range(KD):
                    w2b = load_w(tau, W2_r, W2b_ds[m2], m2, MF, "w2r", "w2b")
                    py = psB.tile([128, FT], F32, name="py")
                    for k2 in range(MF):
                        ks = slice(k2 * 128, (k2 + 1) * 128)
                        nc.tensor.matmul(
                            py[:], lhsT=w2b[:, ks], rhs=h_t[:, k2, :],
                            start=(k2 == 0), stop=(k2 == MF - 1),
                        )
                    osb = op_.tile([128, FT], F32, name="osb")
                    nc.vector.scalar_tensor_tensor(
                        out=osb[:], in0=py[:], scalar=b2T[:, m2 : m2 + 1],
                        in1=cbBs[tau][:], op0=OP.add, op1=OP.mult,
                    )
                    j, jcol = divmod(tau, NFT // NRS)
                    nc.sync.dma_start(
                        out=outT_ds[j].ap()[
                            m2 * 128 : (m2 + 1) * 128,
                            jcol * FT : (jcol + 1) * FT,
                        ],
                        in_=osb[:],
                    )

                if (tau + 1) % (NFT // NRS) == 0:
                    j = tau // (NFT // NRS)
                    nc.gpsimd.collective_compute(
                        "ReduceScatter",
                        OP.add,
                        replica_groups=[CORE_IDS],
                        ins=[outT_ds[j].ap()],
                        outs=[rs_ds[j].ap()],
                    )
                    nc.scalar.dma_start(
                        out=out_ext.ap()[:, j * RSTOK : (j + 1) * RSTOK],
                        in_=rs_ds[j].ap(),
                    )
        const_ctx.close()

    nc.compile()
    return nc


def _get_program():
    global _CACHED
    if _CACHED is None:
        _CACHED = _build()
    return _CACHED


def kernel(
    x,
    padding_mask,
    rms_w,
    router_w,
    router_b,
    W1,
    b1,
    W2,
    b2,
    W3,
    b3,
):
    global LAST_EXEC_NS
    nc = _get_program()

    xf = np.ascontiguousarray(np.asarray(x, np.float32).reshape(NTOK, D))
    shared = {
        "x": xf,
        "rms_w": np.ascontiguousarray(np.asarray(rms_w, np.float32)),
        "router_w": np.ascontiguousarray(np.asarray(router_w, np.float32)),
        "router_b": np.ascontiguousarray(np.asarray(router_b, np.float32)),
    }
    in_maps = []
    for c in CORE_IDS:
        esel = np.zeros([E], np.float32)
        esel[c] = 1.0
        in_maps.append(
            dict(
                shared,
                esel=esel,
                W1=np.ascontiguousarray(np.asarray(W1[c], np.float32)),
                b1=np.ascontiguousarray(np.asarray(b1[c], np.float32)),
                W2=np.ascontiguousarray(np.asarray(W2[c], np.float32)),
                b2=np.ascontiguousarray(np.asarray(b2[c], np.float32)),
                W3=np.ascontiguousarray(np.asarray(W3[c], np.float32)),
                b3=np.ascontiguousarray(np.asarray(b3[c], np.float32)),
            )
        )

    trace = bool(int(os.environ.get("BASSMOE_TRACE", "0")))
    res = run_bass_kernel_spmd(nc, in_maps, CORE_IDS, trace=trace)
    LAST_EXEC_NS = res.exec_time_ns

    outT = np.concatenate([res.results[c]["outp"] for c in CORE_IDS], axis=0)
    out = np.ascontiguousarray(outT.T).reshape(B, T, D)
    aux = np.asarray(0.0, dtype=np.float32)
    return out, aux


# revision 27
# speedup vs baseline: 1.0025x; 1.0025x over previous
"""MoE layer (8 experts, top-2, SwiGLU FFN) on 8 Trainium2 NeuronCores.

Sharding: expert-parallel. Core c holds expert c's weights and computes
  partial_c[d, t] = combine[t, c] * FFN_c(xn)[t, d]   (transposed layout)
for all 4096 tokens; a ReduceScatter(add) over the 8 cores then splits the
summed transposed output row-wise, and the host concatenates + transposes.

On-core pipeline (activations kept d-major, i.e. transposed, so weight
matrices act as pre-transposed stationary operands):
  1. RMSNorm + transpose fused: xn^T chunk = x_tile.T @ diag(1/rms) scaled
     by rms_w per-partition on PSUM eviction (PE transpose trick).
  2. Router in fp32 on PE; top-2 via two masked max-reductions; renorm
     weights w1 = sigmoid(l1 - l2), w2 = 1 - w1 (equivalent to softmax
     top-2 renormalization); combine column broadcast across partitions
     with a ones @ diag(c) matmul.
  3. SwiGLU FFN in bf16 (fp32 PSUM accumulation), biases fused into
     ACT/DVE eviction ops.
"""

import contextlib
import ctypes
import os
import sys
import types

import numpy as np

# ---------------------------------------------------------------------------
# Optional NTFF profiling shim: antenv.axon_hooks is missing in this image;
# recreate it around libaxon's C ABI so trace=True can report HW exec time.
# ---------------------------------------------------------------------------


def _install_axon_hooks_shim(so_path="/opt/axon/libaxon_pjrt.so"):
    if "antenv.axon_hooks" in sys.modules:
        return
    mod = types.ModuleType("antenv.axon_hooks")
    mod._hook = None

    def set_axon_ntff_profile_hook(h):
        mod._hook = h

    def get_axon_ntff_profile_hook():
        return mod._hook

    mod.set_axon_ntff_profile_hook = set_axon_ntff_profile_hook
    mod.get_axon_ntff_profile_hook = get_axon_ntff_profile_hook
    sys.modules["antenv.axon_hooks"] = mod
    try:
        import antenv

        antenv.axon_hooks = mod
    except ImportError:
        pass
    try:
        lib = ctypes.CDLL(so_path)
    except OSError:
        return
    if not hasattr(lib, "axon_start_nrt_profile"):
        return
    lib.axon_start_nrt_profile.argtypes = [
        ctypes.POINTER(ctypes.c_int64),
        ctypes.c_size_t,
    ]
    lib.axon_start_nrt_profile.restype = ctypes.c_int64
    lib.axon_stop_nrt_profile.argtypes = [ctypes.c_char_p]
    lib.axon_stop_nrt_profile.restype = ctypes.c_int64

    @contextlib.contextmanager
    def _hook(output_dir, device_ids):
        import jax

        jax.devices()
        if device_ids:
            ids = (ctypes.c_int64 * len(device_ids))(*device_ids)
            rc = lib.axon_start_nrt_profile(ids, len(device_ids))
        else:
            rc = lib.axon_start_nrt_profile(None, 0)
        if rc != 0:
            raise RuntimeError(f"axon_start_nrt_profile rc={rc}")
        try:
            yield
        finally:
            n = lib.axon_stop_nrt_profile(str(output_dir).encode())
            print(f"profile: {n} file(s) written to {output_dir}", file=sys.stderr)

    set_axon_ntff_profile_hook(_hook)


_install_axon_hooks_shim()

import concourse.bacc as bacc
import concourse.bass as bass
import concourse.mybir as mybir
import concourse.tile as tile
from concourse.bass_utils import run_bass_kernel_spmd
from concourse.masks import make_identity

AF = mybir.ActivationFunctionType
OP = mybir.AluOpType
F32 = mybir.dt.float32
BF16 = mybir.dt.bfloat16

N_CORES = 8
CORE_IDS = list(range(N_CORES))

B, T, D, F, E = 2, 2048, 1024, 4096, 8
NTOK = B * T            # 4096 tokens
TT = 128                # token tile (phase 1)
NTT = NTOK // TT        # 32
FT = 512                # ffn token tile (phase 2)
NFT = NTOK // FT        # 8
KD = D // 128           # 8 contraction chunks over d
MF = F // 128           # 32 f chunks
EPS = 1e-7
MM_DT = BF16            # FFN matmul dtype

LAST_EXEC_NS = None
_CACHED = None


def _build():
    nc = bacc.Bacc(
        "TRN2", target_bir_lowering=False, debug=False, num_devices=N_CORES
    )

    x_d = nc.dram_tensor("x", [NTOK, D], F32, kind="ExternalInput")
    rmsw_d = nc.dram_tensor("rms_w", [D], F32, kind="ExternalInput")
    rw_d = nc.dram_tensor("router_w", [E, D], F32, kind="ExternalInput")
    rb_d = nc.dram_tensor("router_b", [E], F32, kind="ExternalInput")
    esel_d = nc.dram_tensor("esel", [E], F32, kind="ExternalInput")
    W1_d = nc.dram_tensor("W1", [D, F], F32, kind="ExternalInput")
    b1_d = nc.dram_tensor("b1", [F], F32, kind="ExternalInput")
    W2_d = nc.dram_tensor("W2", [F, D], F32, kind="ExternalInput")
    b2_d = nc.dram_tensor("b2", [D], F32, kind="ExternalInput")
    W3_d = nc.dram_tensor("W3", [D, F], F32, kind="ExternalInput")
    b3_d = nc.dram_tensor("b3", [F], F32, kind="ExternalInput")

    xnb_ds = [
        nc.dram_tensor(f"xnb{i}", [D, FT], BF16) for i in range(NFT)
    ]  # normalized, transposed, bf16, one tensor per ffn supertile
    NRS = int(os.environ.get("BASSMOE_NRS", "8"))  # reduce-scatter chunks
    RSTOK = NTOK // NRS
    outT_ds = [nc.dram_tensor(f"outT{j}", [D, RSTOK], F32) for j in range(NRS)]
    rs_ds = [
        nc.dram_tensor(f"rs{j}", [D // N_CORES, RSTOK], F32) for j in range(NRS)
    ]
    W1b_ds = [nc.dram_tensor(f"W1b{m}", [128, KD * 128], BF16) for m in range(MF)]
    W3b_ds = [nc.dram_tensor(f"W3b{m}", [128, KD * 128], BF16) for m in range(MF)]
    W2b_ds = [nc.dram_tensor(f"W2b{m}", [128, MF * 128], BF16) for m in range(KD)]
    out_ext = nc.dram_tensor("outp", [D // N_CORES, NTOK], F32, kind="ExternalOutput")

    with tile.TileContext(nc) as tc:
        const_ctx = contextlib.ExitStack()
        const = const_ctx.enter_context(tc.tile_pool(name="const", bufs=1))
        with contextlib.ExitStack() as ctx:
            ph1 = ctx.enter_context(tc.tile_pool(name="ph1", bufs=5))
            xnp = ctx.enter_context(tc.tile_pool(name="xnp", bufs=4))
            ps1 = ctx.enter_context(
                tc.tile_pool(name="ps1", bufs=2, space="PSUM")
            )

            # ---- constants ----
            ident = const.tile([128, 128], F32)
            make_identity(nc, ident[:])
            ones_t = const.tile([128, 128], F32)
            nc.vector.memset(ones_t[:], 1.0)

            rw_sb = const.tile([E, D], F32)
            nc.sync.dma_start(out=rw_sb[:], in_=rw_d.ap())
            rwT = const.tile([128, KD, E], F32)
            for k in range(KD):
                pt = ps1.tile([128, 128], F32, name="ps_big")[:, :E]
                nc.tensor.transpose(
                    pt[:], rw_sb[:, k * 128 : (k + 1) * 128], ident[:E, :E]
                )
                nc.vector.tensor_copy(out=rwT[:, k, :], in_=pt[:])

            def load_col_chunks(dram, n, name):
                # [n*128] dram vector -> [128, n] sbuf tile, col j = chunk j
                raw = const.tile([n, 128], F32, name=name + "_raw")
                nc.sync.dma_start(
                    out=raw[:], in_=dram.ap().rearrange("(m p) -> m p", p=128)
                )
                pt = ps1.tile([128, 128], F32, name="ps_big")[:, :n]
                nc.tensor.transpose(pt[:], raw[:], ident[:n, :n])
                out = const.tile([128, n], F32, name=name)
                nc.vector.tensor_copy(out=out[:], in_=pt[:])
                return out

            b1T = load_col_chunks(b1_d, MF, "b1T")
            b3T = load_col_chunks(b3_d, MF, "b3T")
            b2T = load_col_chunks(b2_d, KD, "b2T")
            rmswT = load_col_chunks(rmsw_d, KD, "rmswT")

            rb_row = const.tile([1, E], F32)
            nc.sync.dma_start(
                out=rb_row[:], in_=rb_d.ap().rearrange("(a e) -> a e", a=1)
            )
            rbB = const.tile([128, E], F32)
            nc.gpsimd.partition_broadcast(rbB[:], rb_row[:1, :])
            esel_row = const.tile([1, E], F32)
            nc.sync.dma_start(
                out=esel_row[:], in_=esel_d.ap().rearrange("(a e) -> a e", a=1)
            )
            eselB = const.tile([128, E], F32)
            nc.gpsimd.partition_broadcast(eselB[:], esel_row[:1, :])

            cbBs = [
                const.tile([128, FT], F32, name=f"cbB{i}") for i in range(NFT)
            ]  # combine col bcast over partitions, one tile per ffn supertile

            # ---- phase 1: rmsnorm+transpose, router, combine ----
            # grouped passes (8 tiles each): stats batched per group so the
            # first transposes issue after ~8 tiles instead of all 32
            SSQ = const.tile([128, NTT], F32)
            MSQ = const.tile([128, NTT], F32)
            RMS = const.tile([128, NTT], F32)
            INV = const.tile([128, NTT], F32)
            EQ1 = const.tile([128, NTT, E], F32)
            EQ2 = const.tile([128, NTT, E], F32)
            DLT = const.tile([128, NTT], F32)
            GRP = 8
            for g in range(0, NTT, GRP):
                gs = slice(g, g + GRP)
                for t in range(g, g + GRP):
                    ts = slice(t * TT, (t + 1) * TT)
                    x_t = ph1.tile([128, D], F32, name="x_t")
                    nc.sync.dma_start(out=x_t[:], in_=x_d.ap()[ts, :])
                    sq = ph1.tile([128, D], F32, name="sq")
                    nc.vector.scalar_tensor_tensor(
                        out=sq[:], in0=x_t[:], scalar=1.0, in1=x_t[:],
                        op0=OP.bypass, op1=OP.mult, accum_out=SSQ[:, t : t + 1],
                    )
                nc.vector.tensor_scalar(
                    out=MSQ[:, gs], in0=SSQ[:, gs], scalar1=1.0 / D,
                    scalar2=EPS, op0=OP.mult, op1=OP.add,
                )
                nc.scalar.sqrt(RMS[:, gs], MSQ[:, gs])
                nc.vector.reciprocal(INV[:, gs], RMS[:, gs])

                for t in range(g, g + GRP):
                    ts = slice(t * TT, (t + 1) * TT)
                    x_t = ph1.tile([128, D], F32, name="x_t")
                    nc.sync.dma_start(out=x_t[:], in_=x_d.ap()[ts, :])
                    dg = ph1.tile([128, 128], F32, name="dg")
                    nc.vector.tensor_scalar_mul(
                        dg[:], ident[:], INV[:, t : t + 1]
                    )

                    xn_t = xnp.tile([128, KD, 128], F32, name="xn_t")
                    xnb_t = xnp.tile([128, KD, 128], BF16, name="xnb_t")
                    tau, col = divmod(t, FT // TT)
                    for k in range(KD):
                        pxn = ps1.tile([128, 128], F32, name="ps_big")
                        nc.tensor.matmul(
                            pxn[:], lhsT=x_t[:, k * 128 : (k + 1) * 128],
                            rhs=dg[:], start=True, stop=True,
                        )
                        nc.vector.tensor_scalar_mul(
                            xn_t[:, k, :], pxn[:], rmswT[:, k : k + 1]
                        )
                        nc.vector.tensor_copy(
                            out=xnb_t[:, k, :], in_=xn_t[:, k, :]
                        )
                        nc.sync.dma_start(
                            out=xnb_ds[tau].ap()[
                                k * 128 : (k + 1) * 128,
                                col * TT : (col + 1) * TT,
                            ],
                            in_=xnb_t[:, k, :],
                        )

                    lg_ps = ps1.tile([128, 128], F32, name="ps_big")[:, :E]
                    for k in range(KD):
                        nc.tensor.matmul(
                            lg_ps[:], lhsT=xn_t[:, k, :], rhs=rwT[:, k, :],
                            start=(k == 0), stop=(k == KD - 1),
                        )
                    lg = ph1.tile([128, E], F32, name="lg")
                    nc.vector.tensor_add(out=lg[:], in0=lg_ps[:], in1=rbB[:])
                    l1 = ph1.tile([128, 1], F32, name="l1")
                    nc.vector.tensor_reduce(
                        l1[:], lg[:], axis=mybir.AxisListType.X, op=OP.max
                    )
                    nc.vector.tensor_scalar(
                        out=EQ1[:, t, :], in0=lg[:], scalar1=l1[:],
                        scalar2=None, op0=OP.is_equal,
                    )
                    lg2 = ph1.tile([128, E], F32, name="lg2")
                    nc.vector.scalar_tensor_tensor(
                        out=lg2[:], in0=EQ1[:, t, :], scalar=-1e30, in1=lg[:],
                        op0=OP.mult, op1=OP.add,
                    )
                    l2 = ph1.tile([128, 1], F32, name="l2")
                    nc.vector.tensor_reduce(
                        l2[:], lg2[:], axis=mybir.AxisListType.X, op=OP.max
                    )
                    nc.vector.tensor_scalar(
                        out=EQ2[:, t, :], in0=lg2[:], scalar1=l2[:],
                        scalar2=None, op0=OP.is_equal,
                    )
                    nc.vector.tensor_sub(
                        out=DLT[:, t : t + 1], in0=l1[:], in1=l2[:]
                    )

            # pass C: batched sigmoid for the top-2 renorm weights
            WA = const.tile([128, NTT], F32)
            nc.scalar.activation(WA[:], DLT[:], AF.Sigmoid)
            WB = const.tile([128, NTT], F32)
            nc.vector.tensor_scalar(
                out=WB[:], in0=WA[:], scalar1=-1.0, scalar2=1.0,
                op0=OP.mult, op1=OP.add,
            )

            # pass D: combine column for this expert, broadcast over partitions
            for t in range(NTT):
                tau, col = divmod(t, FT // TT)
                tmp = ph1.tile([128, E], F32, name="tmp")
                nc.vector.tensor_scalar_mul(tmp[:], EQ2[:, t, :], WB[:, t : t + 1])
                cmb = ph1.tile([128, E], F32, name="cmb")
                nc.vector.scalar_tensor_tensor(
                    out=cmb[:], in0=EQ1[:, t, :], scalar=WA[:, t : t + 1],
                    in1=tmp[:], op0=OP.mult, op1=OP.add,
                )
                cmb2 = ph1.tile([128, E], F32, name="cmb2")
                c_col = ph1.tile([128, 1], F32, name="c_col")
                nc.vector.scalar_tensor_tensor(
                    out=cmb2[:], in0=cmb[:], scalar=1.0, in1=eselB[:],
                    op0=OP.bypass, op1=OP.mult, accum_out=c_col[:],
                )
                dgc = ph1.tile([128, 128], F32, name="dgc")
                nc.vector.tensor_scalar_mul(dgc[:], ident[:], c_col[:])
                cb_ps = ps1.tile([128, 128], F32, name="ps_big")
                nc.tensor.matmul(
                    cb_ps[:], lhsT=ones_t[:], rhs=dgc[:], start=True, stop=True
                )
                nc.vector.tensor_copy(
                    out=cbBs[tau][:, col * TT : (col + 1) * TT], in_=cb_ps[:]
                )

        # ---- phase 2: SwiGLU FFN in bf16 ----
        with contextlib.ExitStack() as ctx:
            xn2 = ctx.enter_context(tc.tile_pool(name="xn2", bufs=3))
            wp = ctx.enter_context(tc.tile_pool(name="wp", bufs=5))
            wpr = ctx.enter_context(tc.tile_pool(name="wpr", bufs=2))
            hp = ctx.enter_context(tc.tile_pool(name="hp", bufs=1))
            op_ = ctx.enter_context(tc.tile_pool(name="op", bufs=4))
            psA = ctx.enter_context(
                tc.tile_pool(name="psA", bufs=2, space="PSUM")
            )
            psB = ctx.enter_context(
                tc.tile_pool(name="psB", bufs=2, space="PSUM")
            )

            W1_r = W1_d.ap().rearrange("(k p) f -> p k f", p=128)
            W3_r = W3_d.ap().rearrange("(k p) f -> p k f", p=128)
            W2_r = W2_d.ap().rearrange("(k p) f -> p k f", p=128)

            def load_w(tau, raw_src, cache_d, m, nk, rtag, btag, gw=1):
                # tau 0: load fp32 slice, cast, save bf16 for later taus
                wb = wp.tile([128, nk * 128], MM_DT, name=btag)
                if tau == 0:
                    wr = wpr.tile([128, nk, 128], F32, name=rtag)
                    nc.sync.dma_start(
                        out=wr[:], in_=raw_src[:, :, m * 128 : (m + 1) * 128]
                    )
                    nc.vector.tensor_copy(
                        out=wb[:], in_=wr[:].rearrange("p k c -> p (k c)")
                    )
                    nc.sync.dma_start(out=cache_d.ap(), in_=wb[:])
                else:
                    nc.sync.dma_start(out=wb[:], in_=cache_d.ap())
                return wb

            for tau in range(NFT):
                xb = xn2.tile([128, KD, FT], MM_DT, name="xb")
                nc.sync.dma_start(
                    out=xb[:],
                    in_=xnb_ds[tau].ap().rearrange("(k p) n -> p k n", p=128),
                )

                h_t = hp.tile([128, MF, FT], MM_DT, name="h_t")
                for m in range(MF):
                    w1b = load_w(tau, W1_r, W1b_ds[m], m, KD, "w1r", "w1b", gw=2)
                    w3b = load_w(tau, W3_r, W3b_ds[m], m, KD, "w3r", "w3b", gw=2)

                    p1 = psA.tile([128, FT], F32, name="p1")
                    p3 = psA.tile([128, FT], F32, name="p3")
                    for k in range(KD):
                        ks = slice(k * 128, (k + 1) * 128)
                        nc.tensor.matmul(
                            p1[:], lhsT=w1b[:, ks], rhs=xb[:, k, :],
                            start=(k == 0), stop=(k == KD - 1),
                        )
                    for k in range(KD):
                        ks = slice(k * 128, (k + 1) * 128)
                        nc.tensor.matmul(
                            p3[:], lhsT=w3b[:, ks], rhs=xb[:, k, :],
                            start=(k == 0), stop=(k == KD - 1),
                        )
                    h1s = op_.tile([128, FT], MM_DT, name="h1s")
                    nc.scalar.activation(
                        h1s[:], p1[:], AF.Silu, bias=b1T[:, m : m + 1]
                    )
                    nc.vector.scalar_tensor_tensor(
                        out=h_t[:, m, :], in0=p3[:], scalar=b3T[:, m : m + 1],
                        in1=h1s[:], op0=OP.add, op1=OP.mult,
                    )

                for m2 in range(KD):
                    w2b = load_w(tau, W2_r, W2b_ds[m2], m2, MF, "w2r", "w2b")
                    py = psB.tile([128, FT], F32, name="py")
                    for k2 in range(MF):
                        ks = slice(k2 * 128, (k2 + 1) * 128)
                        nc.tensor.matmul(
                            py[:], lhsT=w2b[:, ks], rhs=h_t[:, k2, :],
                            start=(k2 == 0), stop=(k2 == MF - 1),
                        )
                    osb = op_.tile([128, FT], F32, name="osb")
                    nc.vector.scalar_tensor_tensor(
                        out=osb[:], in0=py[:], scalar=b2T[:, m2 : m2 + 1],
                        in1=cbBs[tau][:], op0=OP.add, op1=OP.mult,
                    )
                    j, jcol = divmod(tau, NFT // NRS)
                    nc.sync.dma_start(
                        out=outT_ds[j].ap()[
                            m2 * 128 : (m2 + 1) * 128,
                            jcol * FT : (jcol + 1) * FT,
                        ],
                        in_=osb[:],
                    )

                if (tau + 1) % (NFT // NRS) == 0:
                    j = tau // (NFT // NRS)
                    nc.gpsimd.collective_compute(
                        "ReduceScatter",
                        OP.add,
                        replica_groups=[CORE_IDS],
                        ins=[outT_ds[j].ap()],
                        outs=[rs_ds[j].ap()],
                    )
                    nc.scalar.dma_start(
                        out=out_ext.ap()[:, j * RSTOK : (j + 1) * RSTOK],
                        in_=rs_ds[j].ap(),
                    )
        const_ctx.close()

    nc.compile()
    return nc


def _get_program():
    global _CACHED
    if _CACHED is None:
        _CACHED = _build()
    return _CACHED


def kernel(
    x,
    padding_mask,
    rms_w,
    router_w,
    router_b,
    W1,
    b1,
    W2,
    b2,
    W3,
    b3,
):
    global LAST_EXEC_NS
    nc = _get_program()

    xf = np.ascontiguousarray(np.asarray(x, np.float32).reshape(NTOK, D))
    shared = {
        "x": xf,
        "rms_w": np.ascontiguousarray(np.asarray(rms_w, np.float32)),
        "router_w": np.ascontiguousarray(np.asarray(router_w, np.float32)),
        "router_b": np.ascontiguousarray(np.asarray(router_b, np.float32)),
    }
    in_maps = []
    for c in CORE_IDS:
        esel = np.zeros([E], np.float32)
        esel[c] = 1.0
        in_maps.append(
            dict(
                shared,
                esel=esel,
                W1=np.ascontiguousarray(np.asarray(W1[c], np.float32)),
                b1=np.ascontiguousarray(np.asarray(b1[c], np.float32)),
                W2=np.ascontiguousarray(np.asarray(W2[c], np.float32)),
                b2=np.ascontiguousarray(np.asarray(b2[c], np.float32)),
                W3=np.ascontiguousarray(np.asarray(W3[c], np.float32)),
                b3=np.ascontiguousarray(np.asarray(b3[c], np.float32)),
            )
        )

    trace = bool(int(os.environ.get("BASSMOE_TRACE", "0")))
    res = run_bass_kernel_spmd(nc, in_maps, CORE_IDS, trace=trace)
    LAST_EXEC_NS = res.exec_time_ns

    outT = np.concatenate([res.results[c]["outp"] for c in CORE_IDS], axis=0)
    out = np.ascontiguousarray(outT.T).reshape(B, T, D)
    aux = np.asarray(0.0, dtype=np.float32)
    return out, aux


# revision 28
# speedup vs baseline: 1.0087x; 1.0061x over previous
"""MoE layer (8 experts, top-2, SwiGLU FFN) on 8 Trainium2 NeuronCores.

Sharding: expert-parallel. Core c holds expert c's weights and computes
  partial_c[d, t] = combine[t, c] * FFN_c(xn)[t, d]   (transposed layout)
for all 4096 tokens; a ReduceScatter(add) over the 8 cores then splits the
summed transposed output row-wise, and the host concatenates + transposes.

On-core pipeline (activations kept d-major, i.e. transposed, so weight
matrices act as pre-transposed stationary operands):
  1. RMSNorm + transpose fused: xn^T chunk = x_tile.T @ diag(1/rms) scaled
     by rms_w per-partition on PSUM eviction (PE transpose trick).
  2. Router in fp32 on PE; top-2 via two masked max-reductions; renorm
     weights w1 = sigmoid(l1 - l2), w2 = 1 - w1 (equivalent to softmax
     top-2 renormalization); combine column broadcast across partitions
     with a ones @ diag(c) matmul.
  3. SwiGLU FFN in bf16 (fp32 PSUM accumulation), biases fused into
     ACT/DVE eviction ops.
"""

import contextlib
import ctypes
import os
import sys
import types

import numpy as np

# ---------------------------------------------------------------------------
# Optional NTFF profiling shim: antenv.axon_hooks is missing in this image;
# recreate it around libaxon's C ABI so trace=True can report HW exec time.
# ---------------------------------------------------------------------------


def _install_axon_hooks_shim(so_path="/opt/axon/libaxon_pjrt.so"):
    if "antenv.axon_hooks" in sys.modules:
        return
    mod = types.ModuleType("antenv.axon_hooks")
    mod._hook = None

    def set_axon_ntff_profile_hook(h):
        mod._hook = h

    def get_axon_ntff_profile_hook():
        return mod._hook

    mod.set_axon_ntff_profile_hook = set_axon_ntff_profile_hook
    mod.get_axon_ntff_profile_hook = get_axon_ntff_profile_hook
    sys.modules["antenv.axon_hooks"] = mod
    try:
        import antenv

        antenv.axon_hooks = mod
    except ImportError:
        pass
    try:
        lib = ctypes.CDLL(so_path)
    except OSError:
        return
    if not hasattr(lib, "axon_start_nrt_profile"):
        return
    lib.axon_start_nrt_profile.argtypes = [
        ctypes.POINTER(ctypes.c_int64),
        ctypes.c_size_t,
    ]
    lib.axon_start_nrt_profile.restype = ctypes.c_int64
    lib.axon_stop_nrt_profile.argtypes = [ctypes.c_char_p]
    lib.axon_stop_nrt_profile.restype = ctypes.c_int64

    @contextlib.contextmanager
    def _hook(output_dir, device_ids):
        import jax

        jax.devices()
        if device_ids:
            ids = (ctypes.c_int64 * len(device_ids))(*device_ids)
            rc = lib.axon_start_nrt_profile(ids, len(device_ids))
        else:
            rc = lib.axon_start_nrt_profile(None, 0)
        if rc != 0:
            raise RuntimeError(f"axon_start_nrt_profile rc={rc}")
        try:
            yield
        finally:
            n = lib.axon_stop_nrt_profile(str(output_dir).encode())
            print(f"profile: {n} file(s) written to {output_dir}", file=sys.stderr)

    set_axon_ntff_profile_hook(_hook)


_install_axon_hooks_shim()

import concourse.bacc as bacc
import concourse.bass as bass
import concourse.mybir as mybir
import concourse.tile as tile
from concourse.bass_utils import run_bass_kernel_spmd
from concourse.masks import make_identity

AF = mybir.ActivationFunctionType
OP = mybir.AluOpType
F32 = mybir.dt.float32
BF16 = mybir.dt.bfloat16

N_CORES = 8
CORE_IDS = list(range(N_CORES))

B, T, D, F, E = 2, 2048, 1024, 4096, 8
NTOK = B * T            # 4096 tokens
TT = 128                # token tile (phase 1)
NTT = NTOK // TT        # 32
FT = 512                # ffn token tile (phase 2)
NFT = NTOK // FT        # 8
KD = D // 128           # 8 contraction chunks over d
MF = F // 128           # 32 f chunks
EPS = 1e-7
MM_DT = BF16            # FFN matmul dtype

LAST_EXEC_NS = None
_CACHED = None


def _build():
    nc = bacc.Bacc(
        "TRN2", target_bir_lowering=False, debug=False, num_devices=N_CORES
    )

    x_d = nc.dram_tensor("x", [NTOK, D], F32, kind="ExternalInput")
    rmsw_d = nc.dram_tensor("rms_w", [D], F32, kind="ExternalInput")
    rw_d = nc.dram_tensor("router_w", [E, D], F32, kind="ExternalInput")
    rb_d = nc.dram_tensor("router_b", [E], F32, kind="ExternalInput")
    esel_d = nc.dram_tensor("esel", [E], F32, kind="ExternalInput")
    W1_d = nc.dram_tensor("W1", [D, F], F32, kind="ExternalInput")
    b1_d = nc.dram_tensor("b1", [F], F32, kind="ExternalInput")
    W2_d = nc.dram_tensor("W2", [F, D], F32, kind="ExternalInput")
    b2_d = nc.dram_tensor("b2", [D], F32, kind="ExternalInput")
    W3_d = nc.dram_tensor("W3", [D, F], F32, kind="ExternalInput")
    b3_d = nc.dram_tensor("b3", [F], F32, kind="ExternalInput")

    xnb_ds = [
        nc.dram_tensor(f"xnb{i}", [D, FT], BF16) for i in range(NFT)
    ]  # normalized, transposed, bf16, one tensor per ffn supertile
    NRS = int(os.environ.get("BASSMOE_NRS", "8"))  # reduce-scatter chunks
    RSTOK = NTOK // NRS
    outT_ds = [nc.dram_tensor(f"outT{j}", [D, RSTOK], F32) for j in range(NRS)]
    rs_ds = [
        nc.dram_tensor(f"rs{j}", [D // N_CORES, RSTOK], F32) for j in range(NRS)
    ]
    W1b_ds = [nc.dram_tensor(f"W1b{m}", [128, KD * 128], BF16) for m in range(MF)]
    W3b_ds = [nc.dram_tensor(f"W3b{m}", [128, KD * 128], BF16) for m in range(MF)]
    W2b_ds = [nc.dram_tensor(f"W2b{m}", [128, MF * 128], BF16) for m in range(KD)]
    out_ext = nc.dram_tensor("outp", [D // N_CORES, NTOK], F32, kind="ExternalOutput")

    with tile.TileContext(nc) as tc:
        const_ctx = contextlib.ExitStack()
        const = const_ctx.enter_context(tc.tile_pool(name="const", bufs=1))
        with contextlib.ExitStack() as ctx:
            ph1 = ctx.enter_context(tc.tile_pool(name="ph1", bufs=5))
            xnp = ctx.enter_context(tc.tile_pool(name="xnp", bufs=4))
            ps1 = ctx.enter_context(
                tc.tile_pool(name="ps1", bufs=2, space="PSUM")
            )

            # ---- constants ----
            ident = const.tile([128, 128], F32)
            make_identity(nc, ident[:])
            ones_t = const.tile([128, 128], F32)
            nc.vector.memset(ones_t[:], 1.0)

            rw_sb = const.tile([E, D], F32)
            nc.sync.dma_start(out=rw_sb[:], in_=rw_d.ap())
            rwT = const.tile([128, KD, E], F32)
            for k in range(KD):
                pt = ps1.tile([128, 128], F32, name="ps_big")[:, :E]
                nc.tensor.transpose(
                    pt[:], rw_sb[:, k * 128 : (k + 1) * 128], ident[:E, :E]
                )
                nc.vector.tensor_copy(out=rwT[:, k, :], in_=pt[:])

            def load_col_chunks(dram, n, name):
                # [n*128] dram vector -> [128, n] sbuf tile, col j = chunk j
                raw = const.tile([n, 128], F32, name=name + "_raw")
                nc.sync.dma_start(
                    out=raw[:], in_=dram.ap().rearrange("(m p) -> m p", p=128)
                )
                pt = ps1.tile([128, 128], F32, name="ps_big")[:, :n]
                nc.tensor.transpose(pt[:], raw[:], ident[:n, :n])
                out = const.tile([128, n], F32, name=name)
                nc.vector.tensor_copy(out=out[:], in_=pt[:])
                return out

            b1T = load_col_chunks(b1_d, MF, "b1T")
            b3T = load_col_chunks(b3_d, MF, "b3T")
            b2T = load_col_chunks(b2_d, KD, "b2T")
            rmswT = load_col_chunks(rmsw_d, KD, "rmswT")

            rb_row = const.tile([1, E], F32)
            nc.sync.dma_start(
                out=rb_row[:], in_=rb_d.ap().rearrange("(a e) -> a e", a=1)
            )
            rbB = const.tile([128, E], F32)
            nc.gpsimd.partition_broadcast(rbB[:], rb_row[:1, :])
            esel_row = const.tile([1, E], F32)
            nc.sync.dma_start(
                out=esel_row[:], in_=esel_d.ap().rearrange("(a e) -> a e", a=1)
            )
            eselB = const.tile([128, E], F32)
            nc.gpsimd.partition_broadcast(eselB[:], esel_row[:1, :])

            cbBs = [
                const.tile([128, FT], F32, name=f"cbB{i}") for i in range(NFT)
            ]  # combine col bcast over partitions, one tile per ffn supertile

            # ---- phase 1: rmsnorm+transpose, router, combine ----
            # pass A: sum-of-squares for all tiles, then one batched
            # sqrt/reciprocal (avoids per-tile ACT table thrash + chains)
            SSQ = const.tile([128, NTT], F32)
            for t in range(NTT):
                ts = slice(t * TT, (t + 1) * TT)
                x_t = ph1.tile([128, D], F32, name="x_t")
                nc.sync.dma_start(out=x_t[:], in_=x_d.ap()[ts, :])
                sq = ph1.tile([128, D], F32, name="sq")
                nc.vector.scalar_tensor_tensor(
                    out=sq[:], in0=x_t[:], scalar=1.0, in1=x_t[:],
                    op0=OP.bypass, op1=OP.mult, accum_out=SSQ[:, t : t + 1],
                )
            MSQ = const.tile([128, NTT], F32)
            nc.vector.tensor_scalar(
                out=MSQ[:], in0=SSQ[:], scalar1=1.0 / D, scalar2=EPS,
                op0=OP.mult, op1=OP.add,
            )
            RMS = const.tile([128, NTT], F32)
            nc.scalar.sqrt(RMS[:], MSQ[:])
            INV = const.tile([128, NTT], F32)
            nc.vector.reciprocal(INV[:], RMS[:])

            # pass B: per tile: transpose+normalize xn, router logits, top-2
            # masks; defer all sigmoid/combine math to batched passes C/D
            EQ1 = const.tile([128, NTT, E], F32)
            EQ2 = const.tile([128, NTT, E], F32)
            DLT = const.tile([128, NTT], F32)
            for t in range(NTT):
                ts = slice(t * TT, (t + 1) * TT)
                x_t = ph1.tile([128, D], F32, name="x_t")
                nc.sync.dma_start(out=x_t[:], in_=x_d.ap()[ts, :])
                dg = ph1.tile([128, 128], F32, name="dg")
                nc.vector.tensor_scalar_mul(dg[:], ident[:], INV[:, t : t + 1])

                xn_t = xnp.tile([128, KD, 128], F32, name="xn_t")
                xnb_t = xnp.tile([128, KD, 128], BF16, name="xnb_t")
                tau, col = divmod(t, FT // TT)
                for k in range(KD):
                    pxn = ps1.tile([128, 128], F32, name="ps_big")
                    nc.tensor.matmul(
                        pxn[:], lhsT=x_t[:, k * 128 : (k + 1) * 128], rhs=dg[:],
                        start=True, stop=True,
                    )
                    nc.vector.tensor_scalar_mul(
                        xn_t[:, k, :], pxn[:], rmswT[:, k : k + 1]
                    )
                    nc.vector.tensor_copy(out=xnb_t[:, k, :], in_=xn_t[:, k, :])
                    nc.sync.dma_start(
                        out=xnb_ds[tau].ap()[
                            k * 128 : (k + 1) * 128, col * TT : (col + 1) * TT
                        ],
                        in_=xnb_t[:, k, :],
                    )

                lg_ps = ps1.tile([128, 128], F32, name="ps_big")[:, :E]
                for k in range(KD):
                    nc.tensor.matmul(
                        lg_ps[:], lhsT=xn_t[:, k, :], rhs=rwT[:, k, :],
                        start=(k == 0), stop=(k == KD - 1),
                    )
                lg = ph1.tile([128, E], F32, name="lg")
                nc.vector.tensor_add(out=lg[:], in0=lg_ps[:], in1=rbB[:])
                l1 = ph1.tile([128, 1], F32, name="l1")
                nc.vector.tensor_reduce(
                    l1[:], lg[:], axis=mybir.AxisListType.X, op=OP.max
                )
                nc.vector.tensor_scalar(
                    out=EQ1[:, t, :], in0=lg[:], scalar1=l1[:], scalar2=None,
                    op0=OP.is_equal,
                )
                lg2 = ph1.tile([128, E], F32, name="lg2")
                nc.vector.scalar_tensor_tensor(
                    out=lg2[:], in0=EQ1[:, t, :], scalar=-1e30, in1=lg[:],
                    op0=OP.mult, op1=OP.add,
                )
                l2 = ph1.tile([128, 1], F32, name="l2")
                nc.vector.tensor_reduce(
                    l2[:], lg2[:], axis=mybir.AxisListType.X, op=OP.max
                )
                nc.vector.tensor_scalar(
                    out=EQ2[:, t, :], in0=lg2[:], scalar1=l2[:], scalar2=None,
                    op0=OP.is_equal,
                )
                nc.vector.tensor_sub(
                    out=DLT[:, t : t + 1], in0=l1[:], in1=l2[:]
                )

            # pass C: batched sigmoid for the top-2 renorm weights
            WA = const.tile([128, NTT], F32)
            nc.scalar.activation(WA[:], DLT[:], AF.Sigmoid)
            WB = const.tile([128, NTT], F32)
            nc.vector.tensor_scalar(
                out=WB[:], in0=WA[:], scalar1=-1.0, scalar2=1.0,
                op0=OP.mult, op1=OP.add,
            )

            # pass D: combine column for this expert, broadcast over partitions
            for t in range(NTT):
                tau, col = divmod(t, FT // TT)
                tmp = ph1.tile([128, E], F32, name="tmp")
                nc.vector.tensor_scalar_mul(tmp[:], EQ2[:, t, :], WB[:, t : t + 1])
                cmb = ph1.tile([128, E], F32, name="cmb")
                nc.vector.scalar_tensor_tensor(
                    out=cmb[:], in0=EQ1[:, t, :], scalar=WA[:, t : t + 1],
                    in1=tmp[:], op0=OP.mult, op1=OP.add,
                )
                cmb2 = ph1.tile([128, E], F32, name="cmb2")
                c_col = ph1.tile([128, 1], F32, name="c_col")
                nc.vector.scalar_tensor_tensor(
                    out=cmb2[:], in0=cmb[:], scalar=1.0, in1=eselB[:],
                    op0=OP.bypass, op1=OP.mult, accum_out=c_col[:],
                )
                dgc = ph1.tile([128, 128], F32, name="dgc")
                nc.vector.tensor_scalar_mul(dgc[:], ident[:], c_col[:])
                cb_ps = ps1.tile([128, 128], F32, name="ps_big")
                nc.tensor.matmul(
                    cb_ps[:], lhsT=ones_t[:], rhs=dgc[:], start=True, stop=True
                )
                nc.vector.tensor_copy(
                    out=cbBs[tau][:, col * TT : (col + 1) * TT], in_=cb_ps[:]
                )

        # ---- phase 2: SwiGLU FFN in bf16 ----
        with contextlib.ExitStack() as ctx:
            xn2 = ctx.enter_context(tc.tile_pool(name="xn2", bufs=3))
            wp = ctx.enter_context(tc.tile_pool(name="wp", bufs=5))
            wpr = ctx.enter_context(tc.tile_pool(name="wpr", bufs=2))
            hp = ctx.enter_context(tc.tile_pool(name="hp", bufs=1))
            op_ = ctx.enter_context(tc.tile_pool(name="op", bufs=4))
            psA = ctx.enter_context(
                tc.tile_pool(name="psA", bufs=2, space="PSUM")
            )
            psB = ctx.enter_context(
                tc.tile_pool(name="psB", bufs=2, space="PSUM")
            )

            W1_r = W1_d.ap().rearrange("(k p) f -> p k f", p=128)
            W3_r = W3_d.ap().rearrange("(k p) f -> p k f", p=128)
            W2_r = W2_d.ap().rearrange("(k p) f -> p k f", p=128)

            def load_w(tau, raw_src, cache_d, m, nk, rtag, btag):
                # tau 0: load fp32 slice, cast, save bf16 for later taus
                wb = wp.tile([128, nk * 128], MM_DT, name=btag)
                if tau == 0:
                    wr = wpr.tile([128, nk, 128], F32, name=rtag)
                    nc.sync.dma_start(
                        out=wr[:], in_=raw_src[:, :, m * 128 : (m + 1) * 128]
                    )
                    nc.vector.tensor_copy(
                        out=wb[:], in_=wr[:].rearrange("p k c -> p (k c)")
                    )
                    nc.sync.dma_start(out=cache_d.ap(), in_=wb[:])
                else:
                    nc.sync.dma_start(out=wb[:], in_=cache_d.ap())
                return wb

            for tau in range(NFT):
                xb = xn2.tile([128, KD, FT], MM_DT, name="xb")
                nc.sync.dma_start(
                    out=xb[:],
                    in_=xnb_ds[tau].ap().rearrange("(k p) n -> p k n", p=128),
                )

                h_t = hp.tile([128, MF, FT], MM_DT, name="h_t")
                for m in range(MF):
                    w1b = load_w(tau, W1_r, W1b_ds[m], m, KD, "w1r", "w1b")
                    w3b = load_w(tau, W3_r, W3b_ds[m], m, KD, "w3r", "w3b")

                    p1 = psA.tile([128, FT], F32, name="p1")
                    p3 = psA.tile([128, FT], F32, name="p3")
                    for k in range(KD):
                        ks = slice(k * 128, (k + 1) * 128)
                        nc.tensor.matmul(
                            p1[:], lhsT=w1b[:, ks], rhs=xb[:, k, :],
                            start=(k == 0), stop=(k == KD - 1),
                        )
                    for k in range(KD):
                        ks = slice(k * 128, (k + 1) * 128)
                        nc.tensor.matmul(
                            p3[:], lhsT=w3b[:, ks], rhs=xb[:, k, :],
                            start=(k == 0), stop=(k == KD - 1),
                        )
                    h1s = op_.tile([128, FT], MM_DT, name="h1s")
                    nc.scalar.activation(
                        h1s[:], p1[:], AF.Silu, bias=b1T[:, m : m + 1]
                    )
                    nc.vector.scalar_tensor_tensor(
                        out=h_t[:, m, :], in0=p3[:], scalar=b3T[:, m : m + 1],
                        in1=h1s[:], op0=OP.add, op1=OP.mult,
                    )

                for m2 in range(KD):
                    w2b = load_w(tau, W2_r, W2b_ds[m2], m2, MF, "w2r", "w2b")
                    py = psB.tile([128, FT], F32, name="py")
                    for k2 in range(MF):
                        ks = slice(k2 * 128, (k2 + 1) * 128)
                        nc.tensor.matmul(
                            py[:], lhsT=w2b[:, ks], rhs=h_t[:, k2, :],
                            start=(k2 == 0), stop=(k2 == MF - 1),
                        )
                    osb = op_.tile([128, FT], F32, name="osb")
                    nc.vector.scalar_tensor_tensor(
                        out=osb[:], in0=py[:], scalar=b2T[:, m2 : m2 + 1],
                        in1=cbBs[tau][:], op0=OP.add, op1=OP.mult,
                    )
                    j, jcol = divmod(tau, NFT // NRS)
                    nc.sync.dma_start(
                        out=outT_ds[j].ap()[
                            m2 * 128 : (m2 + 1) * 128,
                            jcol * FT : (jcol + 1) * FT,
                        ],
                        in_=osb[:],
                    )

                if (tau + 1) % (NFT // NRS) == 0:
                    j = tau // (NFT // NRS)
                    nc.gpsimd.collective_compute(
                        "ReduceScatter",
                        OP.add,
                        replica_groups=[CORE_IDS],
                        ins=[outT_ds[j].ap()],
                        outs=[rs_ds[j].ap()],
                    )
                    nc.scalar.dma_start(
                        out=out_ext.ap()[:, j * RSTOK : (j + 1) * RSTOK],
                        in_=rs_ds[j].ap(),
                    )
        const_ctx.close()

    nc.compile()
    return nc


def _get_program():
    global _CACHED
    if _CACHED is None:
        _CACHED = _build()
    return _CACHED


def kernel(
    x,
    padding_mask,
    rms_w,
    router_w,
    router_b,
    W1,
    b1,
    W2,
    b2,
    W3,
    b3,
):
    global LAST_EXEC_NS
    nc = _get_program()

    xf = np.ascontiguousarray(np.asarray(x, np.float32).reshape(NTOK, D))
    shared = {
        "x": xf,
        "rms_w": np.ascontiguousarray(np.asarray(rms_w, np.float32)),
        "router_w": np.ascontiguousarray(np.asarray(router_w, np.float32)),
        "router_b": np.ascontiguousarray(np.asarray(router_b, np.float32)),
    }
    in_maps = []
    for c in CORE_IDS:
        esel = np.zeros([E], np.float32)
        esel[c] = 1.0
        in_maps.append(
            dict(
                shared,
                esel=esel,
                W1=np.ascontiguousarray(np.asarray(W1[c], np.float32)),
                b1=np.ascontiguousarray(np.asarray(b1[c], np.float32)),
                W2=np.ascontiguousarray(np.asarray(W2[c], np.float32)),
                b2=np.ascontiguousarray(np.asarray(b2[c], np.float32)),
                W3=np.ascontiguousarray(np.asarray(W3[c], np.float32)),
                b3=np.ascontiguousarray(np.asarray(b3[c], np.float32)),
            )
        )

    trace = bool(int(os.environ.get("BASSMOE_TRACE", "0")))
    res = run_bass_kernel_spmd(nc, in_maps, CORE_IDS, trace=trace)
    LAST_EXEC_NS = res.exec_time_ns

    outT = np.concatenate([res.results[c]["outp"] for c in CORE_IDS], axis=0)
    out = np.ascontiguousarray(outT.T).reshape(B, T, D)
    aux = np.asarray(0.0, dtype=np.float32)
    return out, aux


# revision 30
# speedup vs baseline: 1.0502x; 1.0412x over previous
"""MoE layer (8 experts, top-2, SwiGLU FFN) on 8 Trainium2 NeuronCores.

Sharding: expert-parallel. Core c holds expert c's weights and computes
  partial_c[d, t] = combine[t, c] * FFN_c(xn)[t, d]   (transposed layout)
for all 4096 tokens; a ReduceScatter(add) over the 8 cores then splits the
summed transposed output row-wise, and the host concatenates + transposes.

On-core pipeline (activations kept d-major, i.e. transposed, so weight
matrices act as pre-transposed stationary operands):
  1. RMSNorm + transpose fused: xn^T chunk = x_tile.T @ diag(1/rms) scaled
     by rms_w per-partition on PSUM eviction (PE transpose trick).
  2. Router in fp32 on PE; top-2 via two masked max-reductions; renorm
     weights w1 = sigmoid(l1 - l2), w2 = 1 - w1 (equivalent to softmax
     top-2 renormalization); combine column broadcast across partitions
     with a ones @ diag(c) matmul.
  3. SwiGLU FFN in bf16 (fp32 PSUM accumulation), biases fused into
     ACT/DVE eviction ops.
"""

import contextlib
import ctypes
import os
import sys
import types

import numpy as np

# ---------------------------------------------------------------------------
# Optional NTFF profiling shim: antenv.axon_hooks is missing in this image;
# recreate it around libaxon's C ABI so trace=True can report HW exec time.
# ---------------------------------------------------------------------------


def _install_axon_hooks_shim(so_path="/opt/axon/libaxon_pjrt.so"):
    if "antenv.axon_hooks" in sys.modules:
        return
    mod = types.ModuleType("antenv.axon_hooks")
    mod._hook = None

    def set_axon_ntff_profile_hook(h):
        mod._hook = h

    def get_axon_ntff_profile_hook():
        return mod._hook

    mod.set_axon_ntff_profile_hook = set_axon_ntff_profile_hook
    mod.get_axon_ntff_profile_hook = get_axon_ntff_profile_hook
    sys.modules["antenv.axon_hooks"] = mod
    try:
        import antenv

        antenv.axon_hooks = mod
    except ImportError:
        pass
    try:
        lib = ctypes.CDLL(so_path)
    except OSError:
        return
    if not hasattr(lib, "axon_start_nrt_profile"):
        return
    lib.axon_start_nrt_profile.argtypes = [
        ctypes.POINTER(ctypes.c_int64),
        ctypes.c_size_t,
    ]
    lib.axon_start_nrt_profile.restype = ctypes.c_int64
    lib.axon_stop_nrt_profile.argtypes = [ctypes.c_char_p]
    lib.axon_stop_nrt_profile.restype = ctypes.c_int64

    @contextlib.contextmanager
    def _hook(output_dir, device_ids):
        import jax

        jax.devices()
        if device_ids:
            ids = (ctypes.c_int64 * len(device_ids))(*device_ids)
            rc = lib.axon_start_nrt_profile(ids, len(device_ids))
        else:
            rc = lib.axon_start_nrt_profile(None, 0)
        if rc != 0:
            raise RuntimeError(f"axon_start_nrt_profile rc={rc}")
        try:
            yield
        finally:
            n = lib.axon_stop_nrt_profile(str(output_dir).encode())
            print(f"profile: {n} file(s) written to {output_dir}", file=sys.stderr)

    set_axon_ntff_profile_hook(_hook)


_install_axon_hooks_shim()

import concourse.bacc as bacc
import concourse.bass as bass
import concourse.mybir as mybir
import concourse.tile as tile
from concourse.bass_utils import run_bass_kernel_spmd
from concourse.masks import make_identity

AF = mybir.ActivationFunctionType
OP = mybir.AluOpType
F32 = mybir.dt.float32
BF16 = mybir.dt.bfloat16

N_CORES = 8
CORE_IDS = list(range(N_CORES))

B, T, D, F, E = 2, 2048, 1024, 4096, 8
NTOK = B * T            # 4096 tokens
TT = 128                # token tile (phase 1)
NTT = NTOK // TT        # 32
FT = 512                # ffn token tile (phase 2)
NFT = NTOK // FT        # 8
KD = D // 128           # 8 contraction chunks over d
MF = F // 128           # 32 f chunks
EPS = 1e-7
MM_DT = BF16            # FFN matmul dtype

LAST_EXEC_NS = None
_CACHED = None


def _build():
    nc = bacc.Bacc(
        "TRN2", target_bir_lowering=False, debug=False, num_devices=N_CORES
    )

    x_d = nc.dram_tensor("x", [NTOK, D], F32, kind="ExternalInput")
    rmsw_d = nc.dram_tensor("rms_w", [D], F32, kind="ExternalInput")
    rw_d = nc.dram_tensor("router_w", [E, D], F32, kind="ExternalInput")
    rb_d = nc.dram_tensor("router_b", [E], F32, kind="ExternalInput")
    esel_d = nc.dram_tensor("esel", [E], F32, kind="ExternalInput")
    W1_d = nc.dram_tensor("W1", [D, F], F32, kind="ExternalInput")
    b1_d = nc.dram_tensor("b1", [F], F32, kind="ExternalInput")
    W2_d = nc.dram_tensor("W2", [F, D], F32, kind="ExternalInput")
    b2_d = nc.dram_tensor("b2", [D], F32, kind="ExternalInput")
    W3_d = nc.dram_tensor("W3", [D, F], F32, kind="ExternalInput")
    b3_d = nc.dram_tensor("b3", [F], F32, kind="ExternalInput")

    xnb_ds = [
        nc.dram_tensor(f"xnb{i}", [D, FT], BF16) for i in range(NFT)
    ]  # normalized, transposed, bf16, one tensor per ffn supertile
    NRS = int(os.environ.get("BASSMOE_NRS", "8"))  # reduce-scatter chunks
    RSTOK = NTOK // NRS
    outT_ds = [nc.dram_tensor(f"outT{j}", [D, RSTOK], F32) for j in range(NRS)]
    rs_ds = [
        nc.dram_tensor(f"rs{j}", [D // N_CORES, RSTOK], F32) for j in range(NRS)
    ]
    W1b_ds = [nc.dram_tensor(f"W1b{m}", [128, KD * 128], BF16) for m in range(MF)]
    W3b_ds = [nc.dram_tensor(f"W3b{m}", [128, KD * 128], BF16) for m in range(MF)]
    W2b_ds = [nc.dram_tensor(f"W2b{m}", [128, MF * 128], BF16) for m in range(KD)]
    out_ext = nc.dram_tensor("outp", [D // N_CORES, NTOK], F32, kind="ExternalOutput")

    with tile.TileContext(nc) as tc:
        const_ctx = contextlib.ExitStack()
        const = const_ctx.enter_context(tc.tile_pool(name="const", bufs=1))
        with contextlib.ExitStack() as ctx:
            ph1 = ctx.enter_context(tc.tile_pool(name="ph1", bufs=5))
            xnp = ctx.enter_context(tc.tile_pool(name="xnp", bufs=4))
            ps1 = ctx.enter_context(
                tc.tile_pool(name="ps1", bufs=2, space="PSUM")
            )

            # ---- constants ----
            ident = const.tile([128, 128], F32)
            make_identity(nc, ident[:])
            ones_t = const.tile([128, 128], F32)
            nc.vector.memset(ones_t[:], 1.0)

            rw_sb = const.tile([E, D], F32)
            nc.sync.dma_start(out=rw_sb[:], in_=rw_d.ap())
            rwT = const.tile([128, KD, E], F32)
            for k in range(KD):
                pt = ps1.tile([128, 128], F32, name="ps_big")[:, :E]
                nc.tensor.transpose(
                    pt[:], rw_sb[:, k * 128 : (k + 1) * 128], ident[:E, :E]
                )
                nc.vector.tensor_copy(out=rwT[:, k, :], in_=pt[:])

            def load_col_chunks(dram, n, name):
                # [n*128] dram vector -> [128, n] sbuf tile, col j = chunk j
                raw = const.tile([n, 128], F32, name=name + "_raw")
                nc.sync.dma_start(
                    out=raw[:], in_=dram.ap().rearrange("(m p) -> m p", p=128)
                )
                pt = ps1.tile([128, 128], F32, name="ps_big")[:, :n]
                nc.tensor.transpose(pt[:], raw[:], ident[:n, :n])
                out = const.tile([128, n], F32, name=name)
                nc.vector.tensor_copy(out=out[:], in_=pt[:])
                return out

            b1T = load_col_chunks(b1_d, MF, "b1T")
            b3T = load_col_chunks(b3_d, MF, "b3T")
            b2T = load_col_chunks(b2_d, KD, "b2T")
            rmswT = load_col_chunks(rmsw_d, KD, "rmswT")

            rb_row = const.tile([1, E], F32)
            nc.sync.dma_start(
                out=rb_row[:], in_=rb_d.ap().rearrange("(a e) -> a e", a=1)
            )
            rbB = const.tile([128, E], F32)
            nc.gpsimd.partition_broadcast(rbB[:], rb_row[:1, :])
            esel_row = const.tile([1, E], F32)
            nc.sync.dma_start(
                out=esel_row[:], in_=esel_d.ap().rearrange("(a e) -> a e", a=1)
            )
            eselB = const.tile([128, E], F32)
            nc.gpsimd.partition_broadcast(eselB[:], esel_row[:1, :])

            cbBs = [
                const.tile([128, FT], F32, name=f"cbB{i}") for i in range(NFT)
            ]  # combine col bcast over partitions, one tile per ffn supertile

            # ---- phase 1: rmsnorm+transpose, router, combine ----
            # pass A: sum-of-squares for all tiles, then one batched
            # sqrt/reciprocal (avoids per-tile ACT table thrash + chains)
            SSQ = const.tile([128, NTT], F32)
            for t in range(NTT):
                ts = slice(t * TT, (t + 1) * TT)
                x_t = ph1.tile([128, D], F32, name="x_t")
                nc.sync.dma_start(out=x_t[:], in_=x_d.ap()[ts, :])
                sq = ph1.tile([128, D], F32, name="sq")
                nc.vector.scalar_tensor_tensor(
                    out=sq[:], in0=x_t[:], scalar=1.0, in1=x_t[:],
                    op0=OP.bypass, op1=OP.mult, accum_out=SSQ[:, t : t + 1],
                )
            MSQ = const.tile([128, NTT], F32)
            nc.vector.tensor_scalar(
                out=MSQ[:], in0=SSQ[:], scalar1=1.0 / D, scalar2=EPS,
                op0=OP.mult, op1=OP.add,
            )
            RMS = const.tile([128, NTT], F32)
            nc.scalar.sqrt(RMS[:], MSQ[:])
            INV = const.tile([128, NTT], F32)
            nc.vector.reciprocal(INV[:], RMS[:])

            # pass B: per tile: transpose+normalize xn, router logits, top-2
            # masks; defer all sigmoid/combine math to batched passes C/D
            EQ1 = const.tile([128, NTT, E], F32)
            EQ2 = const.tile([128, NTT, E], F32)
            DLT = const.tile([128, NTT], F32)
            for t in range(NTT):
                ts = slice(t * TT, (t + 1) * TT)
                x_t = ph1.tile([128, D], F32, name="x_t")
                nc.sync.dma_start(out=x_t[:], in_=x_d.ap()[ts, :])
                dg = ph1.tile([128, 128], F32, name="dg")
                nc.vector.tensor_scalar_mul(dg[:], ident[:], INV[:, t : t + 1])

                xn_t = xnp.tile([128, KD, 128], F32, name="xn_t")
                xnb_t = xnp.tile([128, KD, 128], BF16, name="xnb_t")
                tau, col = divmod(t, FT // TT)
                for k in range(KD):
                    pxn = ps1.tile([128, 128], F32, name="ps_big")
                    nc.tensor.matmul(
                        pxn[:], lhsT=x_t[:, k * 128 : (k + 1) * 128], rhs=dg[:],
                        start=True, stop=True,
                    )
                    nc.vector.tensor_scalar_mul(
                        xn_t[:, k, :], pxn[:], rmswT[:, k : k + 1]
                    )
                    nc.vector.tensor_copy(out=xnb_t[:, k, :], in_=xn_t[:, k, :])
                    nc.gpsimd.dma_start(
                        out=xnb_ds[tau].ap()[
                            k * 128 : (k + 1) * 128, col * TT : (col + 1) * TT
                        ],
                        in_=xnb_t[:, k, :],
                    )

                lg_ps = ps1.tile([128, 128], F32, name="ps_big")[:, :E]
                for k in range(KD):
                    nc.tensor.matmul(
                        lg_ps[:], lhsT=xn_t[:, k, :], rhs=rwT[:, k, :],
                        start=(k == 0), stop=(k == KD - 1),
                    )
                lg = ph1.tile([128, E], F32, name="lg")
                nc.vector.tensor_add(out=lg[:], in0=lg_ps[:], in1=rbB[:])
                l1 = ph1.tile([128, 1], F32, name="l1")
                nc.vector.tensor_reduce(
                    l1[:], lg[:], axis=mybir.AxisListType.X, op=OP.max
                )
                nc.vector.tensor_scalar(
                    out=EQ1[:, t, :], in0=lg[:], scalar1=l1[:], scalar2=None,
                    op0=OP.is_equal,
                )
                lg2 = ph1.tile([128, E], F32, name="lg2")
                nc.vector.scalar_tensor_tensor(
                    out=lg2[:], in0=EQ1[:, t, :], scalar=-1e30, in1=lg[:],
                    op0=OP.mult, op1=OP.add,
                )
                l2 = ph1.tile([128, 1], F32, name="l2")
                nc.vector.tensor_reduce(
                    l2[:], lg2[:], axis=mybir.AxisListType.X, op=OP.max
                )
                nc.vector.tensor_scalar(
                    out=EQ2[:, t, :], in0=lg2[:], scalar1=l2[:], scalar2=None,
                    op0=OP.is_equal,
                )
                nc.vector.tensor_sub(
                    out=DLT[:, t : t + 1], in0=l1[:], in1=l2[:]
                )

            # pass C: batched sigmoid for the top-2 renorm weights
            WA = const.tile([128, NTT], F32)
            nc.scalar.activation(WA[:], DLT[:], AF.Sigmoid)
            WB = const.tile([128, NTT], F32)
            nc.vector.tensor_scalar(
                out=WB[:], in0=WA[:], scalar1=-1.0, scalar2=1.0,
                op0=OP.mult, op1=OP.add,
            )

            # pass D: combine column for this expert, broadcast over partitions
            for t in range(NTT):
                tau, col = divmod(t, FT // TT)
                tmp = ph1.tile([128, E], F32, name="tmp")
                nc.vector.tensor_scalar_mul(tmp[:], EQ2[:, t, :], WB[:, t : t + 1])
                cmb = ph1.tile([128, E], F32, name="cmb")
                nc.vector.scalar_tensor_tensor(
                    out=cmb[:], in0=EQ1[:, t, :], scalar=WA[:, t : t + 1],
                    in1=tmp[:], op0=OP.mult, op1=OP.add,
                )
                cmb2 = ph1.tile([128, E], F32, name="cmb2")
                c_col = ph1.tile([128, 1], F32, name="c_col")
                nc.vector.scalar_tensor_tensor(
                    out=cmb2[:], in0=cmb[:], scalar=1.0, in1=eselB[:],
                    op0=OP.bypass, op1=OP.mult, accum_out=c_col[:],
                )
                dgc = ph1.tile([128, 128], F32, name="dgc")
                nc.vector.tensor_scalar_mul(dgc[:], ident[:], c_col[:])
                cb_ps = ps1.tile([128, 128], F32, name="ps_big")
                nc.tensor.matmul(
                    cb_ps[:], lhsT=ones_t[:], rhs=dgc[:], start=True, stop=True
                )
                nc.vector.tensor_copy(
                    out=cbBs[tau][:, col * TT : (col + 1) * TT], in_=cb_ps[:]
                )

        # ---- phase 2: SwiGLU FFN in bf16 ----
        with contextlib.ExitStack() as ctx:
            xn2 = ctx.enter_context(tc.tile_pool(name="xn2", bufs=3))
            wp = ctx.enter_context(tc.tile_pool(name="wp", bufs=5))
            wpr = ctx.enter_context(tc.tile_pool(name="wpr", bufs=2))
            hp = ctx.enter_context(tc.tile_pool(name="hp", bufs=1))
            op_ = ctx.enter_context(tc.tile_pool(name="op", bufs=4))
            psA = ctx.enter_context(
                tc.tile_pool(name="psA", bufs=2, space="PSUM")
            )
            psB = ctx.enter_context(
                tc.tile_pool(name="psB", bufs=2, space="PSUM")
            )

            W1_r = W1_d.ap().rearrange("(k p) f -> p k f", p=128)
            W3_r = W3_d.ap().rearrange("(k p) f -> p k f", p=128)
            W2_r = W2_d.ap().rearrange("(k p) f -> p k f", p=128)

            def load_w(tau, raw_src, cache_d, m, nk, rtag, btag):
                # tau 0: load fp32 slice, cast, save bf16 for later taus
                wb = wp.tile([128, nk * 128], MM_DT, name=btag)
                if tau == 0:
                    wr = wpr.tile([128, nk, 128], F32, name=rtag)
                    nc.sync.dma_start(
                        out=wr[:], in_=raw_src[:, :, m * 128 : (m + 1) * 128]
                    )
                    nc.vector.tensor_copy(
                        out=wb[:], in_=wr[:].rearrange("p k c -> p (k c)")
                    )
                    nc.sync.dma_start(out=cache_d.ap(), in_=wb[:])
                else:
                    nc.sync.dma_start(out=wb[:], in_=cache_d.ap())
                return wb

            for tau in range(NFT):
                xb = xn2.tile([128, KD, FT], MM_DT, name="xb")
                nc.sync.dma_start(
                    out=xb[:],
                    in_=xnb_ds[tau].ap().rearrange("(k p) n -> p k n", p=128),
                )

                h_t = hp.tile([128, MF, FT], MM_DT, name="h_t")
                for m in range(MF):
                    w1b = load_w(tau, W1_r, W1b_ds[m], m, KD, "w1r", "w1b")
                    w3b = load_w(tau, W3_r, W3b_ds[m], m, KD, "w3r", "w3b")

                    p1 = psA.tile([128, FT], F32, name="p1")
                    p3 = psA.tile([128, FT], F32, name="p3")
                    for k in range(KD):
                        ks = slice(k * 128, (k + 1) * 128)
                        nc.tensor.matmul(
                            p1[:], lhsT=w1b[:, ks], rhs=xb[:, k, :],
                            start=(k == 0), stop=(k == KD - 1),
                        )
                    for k in range(KD):
                        ks = slice(k * 128, (k + 1) * 128)
                        nc.tensor.matmul(
                            p3[:], lhsT=w3b[:, ks], rhs=xb[:, k, :],
                            start=(k == 0), stop=(k == KD - 1),
                        )
                    h1s = op_.tile([128, FT], MM_DT, name="h1s")
                    nc.scalar.activation(
                        h1s[:], p1[:], AF.Silu, bias=b1T[:, m : m + 1]
                    )
                    nc.vector.scalar_tensor_tensor(
                        out=h_t[:, m, :], in0=p3[:], scalar=b3T[:, m : m + 1],
                        in1=h1s[:], op0=OP.add, op1=OP.mult,
                    )

                for m2 in range(KD):
                    w2b = load_w(tau, W2_r, W2b_ds[m2], m2, MF, "w2r", "w2b")
                    py = psB.tile([128, FT], F32, name="py")
                    for k2 in range(MF):
                        ks = slice(k2 * 128, (k2 + 1) * 128)
                        nc.tensor.matmul(
                            py[:], lhsT=w2b[:, ks], rhs=h_t[:, k2, :],
                            start=(k2 == 0), stop=(k2 == MF - 1),
                        )
                    osb = op_.tile([128, FT], F32, name="osb")
                    nc.vector.scalar_tensor_tensor(
                        out=osb[:], in0=py[:], scalar=b2T[:, m2 : m2 + 1],
                        in1=cbBs[tau][:], op0=OP.add, op1=OP.mult,
                    )
                    j, jcol = divmod(tau, NFT // NRS)
                    nc.sync.dma_start(
                        out=outT_ds[j].ap()[
                            m2 * 128 : (m2 + 1) * 128,
                            jcol * FT : (jcol + 1) * FT,
                        ],
                        in_=osb[:],
                    )

                if (tau + 1) % (NFT // NRS) == 0:
                    j = tau // (NFT // NRS)
                    nc.gpsimd.collective_compute(
                        "ReduceScatter",
                        OP.add,
                        replica_groups=[CORE_IDS],
                        ins=[outT_ds[j].ap()],
                        outs=[rs_ds[j].ap()],
                    )
                    nc.sync.dma_start(
                        out=out_ext.ap()[:, j * RSTOK : (j + 1) * RSTOK],
                        in_=rs_ds[j].ap(),
                    )
        const_ctx.close()

    nc.compile()
    return nc


def _get_program():
    global _CACHED
    if _CACHED is None:
        _CACHED = _build()
    return _CACHED


def kernel(
    x,
    padding_mask,
    rms_w,
    router_w,
    router_b,
    W1,
    b1,
    W2,
    b2,
    W3,
    b3,
):
    global LAST_EXEC_NS
    nc = _get_program()

    xf = np.ascontiguousarray(np.asarray(x, np.float32).reshape(NTOK, D))
    shared = {
        "x": xf,
        "rms_w": np.ascontiguousarray(np.asarray(rms_w, np.float32)),
        "router_w": np.ascontiguousarray(np.asarray(router_w, np.float32)),
        "router_b": np.ascontiguousarray(np.asarray(router_b, np.float32)),
    }
    in_maps = []
    for c in CORE_IDS:
        esel = np.zeros([E], np.float32)
        esel[c] = 1.0
        in_maps.append(
            dict(
                shared,
                esel=esel,
                W1=np.ascontiguousarray(np.asarray(W1[c], np.float32)),
                b1=np.ascontiguousarray(np.asarray(b1[c], np.float32)),
                W2=np.ascontiguousarray(np.asarray(W2[c], np.float32)),
                b2=np.ascontiguousarray(np.asarray(b2[c], np.float32)),
                W3=np.ascontiguousarray(np.asarray(W3[c], np.float32)),
                b3=np.ascontiguousarray(np.asarray(b3[c], np.float32)),
            )
        )

    trace = bool(int(os.environ.get("BASSMOE_TRACE", "0")))
    res = run_bass_kernel_spmd(nc, in_maps, CORE_IDS, trace=trace)
    LAST_EXEC_NS = res.exec_time_ns

    outT = np.concatenate([res.results[c]["outp"] for c in CORE_IDS], axis=0)
    out = np.ascontiguousarray(outT.T).reshape(B, T, D)
    aux = np.asarray(0.0, dtype=np.float32)
    return out, aux
